# revision 7
# baseline (speedup 1.0000x reference)
"""Trainium2 Bass kernel for nn_DecoderBlock_85761906966851.

The reference decoder block's attention einsum ('bhss,bshd->bshd') takes the
DIAGONAL of the attention matrix, so token i only needs
    diag_prob_i[h] = exp(s_ii) / sum_{j<=i} exp(s_ij)
per head.  The kernel therefore computes causal row-sums of exp(QK^T)
(flash-style denominators, fused exp+row-accumulate on the scalar engine),
the diagonal scores via an elementwise q*k partition-block reduction, and a
dense per-token pipeline: out = LN2(LN1(x + diag*V@Wo + bo) -> FFN residual).

Sharding: 8 cores = 2 batches x 4 stride offsets; core (b, p) owns tokens
p::4 of batch b.  The stride-4 interleave equalizes causal work across cores
so a single SPMD program fits all: row-slot a (128 rows) attends keys
[0, 512*(a+1)) with one per-core staircase mask on the last 512-key chunk
(keep col c iff c <= 4m + p), added in PSUM via an identity matmul.
No collectives; k is recomputed per core.
"""

import numpy as np


def to_f32r(a):
    """Round fp32 to fp32r (11-bit mantissa, round half up at bit 12)."""
    b = np.ascontiguousarray(a, dtype=np.float32).view(np.uint32)
    r = ((b.astype(np.uint64) + 0x800) & 0xFFFFF000).astype(np.uint32)
    return r.view(np.float32)


B, S, D, H, FF = 2, 2048, 512, 8, 2048
DK = D // H          # 64
P = 128
NT = 512             # tokens per core
NSLOT = 4
DO = D // P          # 4
KI = D // P          # 4
NFT = FF // P        # 16
EPS = 1e-3
NEG = -1.0e30

TRACE = False
LAST_EXEC_NS = None
_CACHE = {}


def _layernorm(nc, pool, mybir, src, dst, g_b, be_b, eps_t):
    f32 = mybir.dt.float32
    st = pool.tile([P, 6], f32, tag="ln_st")
    nc.vector.bn_stats(out=st, in_=src)
    mv = pool.tile([P, 2], f32, tag="ln_mv")
    nc.vector.bn_aggr(out=mv, in_=st)
    nc.scalar.activation(out=mv[:, 1:2], in_=mv[:, 1:2],
                         func=mybir.ActivationFunctionType.Sqrt, bias=eps_t)
    nc.vector.reciprocal(out=mv[:, 1:2], in_=mv[:, 1:2])
    nc.vector.tensor_scalar(out=dst, in0=src, scalar1=mv[:, 0:1], scalar2=mv[:, 1:2],
                            op0=mybir.AluOpType.subtract, op1=mybir.AluOpType.mult)
    nc.vector.tensor_tensor(dst, dst, g_b, mybir.AluOpType.mult)
    nc.vector.tensor_tensor(dst, dst, be_b, mybir.AluOpType.add)


def _build_nc():
    import concourse.bass as bass
    import concourse.mybir as mybir
    import concourse.tile as tile
    from concourse import bacc
    from concourse.masks import make_identity

    f32 = mybir.dt.float32
    f32r = mybir.dt.float32r
    bf16 = mybir.dt.bfloat16
    Alu = mybir.AluOpType
    Act = mybir.ActivationFunctionType

    nc = bacc.Bacc(None, target_bir_lowering=False, debug=False)

    xT = nc.dram_tensor("xT", [D, S], f32r, kind="ExternalInput")
    xTown = nc.dram_tensor("xTown", [D, NT], f32r, kind="ExternalInput")
    Wq = nc.dram_tensor("Wq", [D, D], f32r, kind="ExternalInput")
    Wk = nc.dram_tensor("Wk", [D, D], f32r, kind="ExternalInput")
    Wv = nc.dram_tensor("Wv", [D, D], f32r, kind="ExternalInput")
    Wo = nc.dram_tensor("Wo", [D, D], f32r, kind="ExternalInput")
    W1 = nc.dram_tensor("W1", [D, FF], f32r, kind="ExternalInput")
    W2 = nc.dram_tensor("W2", [FF, D], f32r, kind="ExternalInput")
    vecs = {n: nc.dram_tensor(n, [D], f32, kind="ExternalInput")
            for n in ["bq", "bk", "bv", "bo", "b2", "g1", "be1", "g2", "be2"]}
    b1v = nc.dram_tensor("b1", [FF], f32, kind="ExternalInput")
    maskv = nc.dram_tensor("mask", [P, 512], f32r, kind="ExternalInput")
    keepv = nc.dram_tensor("keep", [P, NSLOT], f32, kind="ExternalInput")
    oselv = nc.dram_tensor("osel", [P, DO, H], f32r, kind="ExternalInput")
    identv = nc.dram_tensor("identr", [P, P], f32r, kind="ExternalInput")
    outv = nc.dram_tensor("out", [NT, D], f32, kind="ExternalOutput")

    def bcast_row(h, n):
        return bass.AP(tensor=h[:].tensor, offset=h[:].offset,
                       ap=[[0, P], *h[:].ap])

    def r(ap):
        return ap.bitcast(f32r)

    with tile.TileContext(nc) as tc:
        with (
            tc.tile_pool(name="const", bufs=1) as cst,
            tc.tile_pool(name="wgt", bufs=2) as wgt,
            tc.tile_pool(name="persist", bufs=1) as per,
            tc.tile_pool(name="stream", bufs=3) as stream,
            tc.tile_pool(name="xcs", bufs=2) as xcs,
            tc.tile_pool(name="expbuf", bufs=1) as expbuf,
        ):
            # ---------------- constants ----------------
            ident = cst.tile([P, P], f32)
            make_identity(nc, ident)
            ident_r = cst.tile([P, P], f32r)
            nc.sync.dma_start(out=ident_r, in_=identv[:])
            eps_t = cst.tile([P, 1], f32)
            nc.vector.memset(eps_t, EPS)
            mask_t = cst.tile([P, 512], f32r)
            nc.sync.dma_start(out=mask_t, in_=maskv[:])
            keep_t = cst.tile([P, NSLOT], f32)
            nc.sync.dma_start(out=keep_t, in_=keepv[:])
            osel_t = cst.tile([P, DO, H], f32r)
            nc.sync.dma_start(out=osel_t, in_=oselv[:])
            bq_t = cst.tile([P, DO], f32)
            nc.sync.dma_start(out=bq_t, in_=vecs["bq"][:].rearrange("(o p) -> p o", p=P))
            bk_t = cst.tile([P, DO], f32)
            nc.sync.dma_start(out=bk_t, in_=vecs["bk"][:].rearrange("(o p) -> p o", p=P))
            b1_t = cst.tile([P, NFT], f32)
            nc.sync.dma_start(out=b1_t, in_=b1v[:].rearrange("(o p) -> p o", p=P))
            bcasts = {}
            for n in ["bv", "bo", "b2", "g1", "be1", "g2", "be2"]:
                t = cst.tile([P, D], f32, tag=f"bc_{n}")
                nc.sync.dma_start(out=t, in_=bcast_row(vecs[n], D))
                bcasts[n] = t

            # ---------------- persistent tensors ----------------
            wq_t = wgt.tile([P, KI, D], f32r, tag="w")
            nc.sync.dma_start(out=wq_t, in_=Wq[:].rearrange("(o p) n -> p o n", p=P))
            wk_t = wgt.tile([P, KI, D], f32r, tag="w")
            nc.sync.dma_start(out=wk_t, in_=Wk[:].rearrange("(o p) n -> p o n", p=P))
            xTo = per.tile([P, KI, NT], f32r)
            nc.sync.dma_start(out=xTo, in_=xTown[:].rearrange("(o p) n -> p o n", p=P))

            qT = per.tile([P, DO, NT], f32r)
            kTo = per.tile([P, DO, NT], f32)
            kT = per.tile([P, DO, S], f32r)
            v_row = per.tile([P, NSLOT, D], f32)
            x_row = per.tile([P, NSLOT, D], f32)
            r1 = per.tile([P, NSLOT, D], f32)
            xn1 = per.tile([P, NSLOT, D], f32)
            xnT = per.tile([P, KI, NT], f32r)
            denom = per.tile([P, NSLOT, H], f32)
            d3b = per.tile([P, H], f32)
            rden = per.tile([P, NSLOT, H], f32)
            sii_eT = per.tile([H, NT], f32)
            dp = per.tile([P, NSLOT, H], f32)
            qkp = per.tile([P, DO, NT], f32r)
            out_sb = per.tile([P, NSLOT, D], f32)

            xT_re = xT[:].rearrange("(o p) s -> p o s", p=P)

            # ============ phase 1: projections, kT, s_ii ============
            with tc.tile_pool(name="pp", bufs=2, space="PSUM") as pp:
                for do in range(DO):
                    ps = pp.tile([P, NT], f32, tag="pp")
                    for ki in range(KI):
                        nc.tensor.matmul(
                            ps, wq_t[:, ki, do * P:(do + 1) * P], xTo[:, ki, :],
                            start=(ki == 0), stop=(ki == KI - 1))
                    nc.vector.tensor_scalar_add(qT[:, do, :], ps, bq_t[:, do:do + 1])

                for do in range(DO):
                    ps = pp.tile([P, NT], f32, tag="pp")
                    for ki in range(KI):
                        nc.tensor.matmul(
                            ps, wk_t[:, ki, do * P:(do + 1) * P], xTo[:, ki, :],
                            start=(ki == 0), stop=(ki == KI - 1))
                    nc.vector.tensor_scalar_add(kTo[:, do, :], ps, bk_t[:, do:do + 1])

                for ck in range(4):
                    xc = xcs.tile([P, KI, 512], f32r, tag="xc")
                    nc.sync.dma_start(out=xc, in_=xT_re[:, :, ck * 512:(ck + 1) * 512])
                    for do in range(DO):
                        ps = pp.tile([P, 512], f32, tag="pk")
                        for ki in range(KI):
                            nc.tensor.matmul(
                                ps, wk_t[:, ki, do * P:(do + 1) * P], xc[:, ki, :],
                                start=(ki == 0), stop=(ki == KI - 1))
                        nc.vector.tensor_scalar_add(
                            kT[:, do, ck * 512:(ck + 1) * 512], ps, bk_t[:, do:do + 1])

                # s_ii^T = per-head partition-block sums of qT ⊙ kTo
                nc.vector.tensor_tensor(qkp[:], qT[:].bitcast(f32), kTo[:], Alu.mult)
                ps_sii = pp.tile([H, NT], f32, tag="sii")
                for dt in range(DO):
                    nc.tensor.matmul(ps_sii, osel_t[:, dt, :], qkp[:, dt, :],
                                     start=(dt == 0), stop=(dt == DO - 1))
                nc.scalar.activation(sii_eT, ps_sii, Act.Exp)

                # wv/wo loads reuse the wq/wk slots (WAR handled by Tile)
                wv_t = wgt.tile([P, KI, D], f32r, tag="w")
                nc.sync.dma_start(out=wv_t, in_=Wv[:].rearrange("(o p) n -> p o n", p=P))
                wo_t = wgt.tile([P, KI, D], f32r, tag="w")
                nc.sync.dma_start(out=wo_t, in_=Wo[:].rearrange("(o p) n -> p o n", p=P))

                for a in range(NSLOT):
                    ps = pp.tile([P, D], f32, tag="pk")
                    for ki in range(KI):
                        nc.tensor.matmul(
                            ps, xTo[:, ki, a * P:(a + 1) * P], wv_t[:, ki, :],
                            start=(ki == 0), stop=(ki == KI - 1))
                    nc.vector.tensor_tensor(v_row[:, a, :], ps, bcasts["bv"], Alu.add)

                for a in range(NSLOT):
                    psr = pp.tile([P, D], f32r, tag="pkr")
                    for ki in range(KI):
                        nc.tensor.transpose(
                            psr[:, ki * P:(ki + 1) * P], xTo[:, ki, a * P:(a + 1) * P], ident_r)
                    nc.vector.tensor_copy(x_row[:, a, :], psr.bitcast(f32))

            # ============ phase 2: causal exp row-sums ============
            for a in range(NSLOT):
                if a < 3:
                    kw = 512 * (a + 1)
                    with tc.tile_pool(name=f"ps{a}", bufs=2, space="PSUM") as sp:
                        for h in range(H):
                            po, pr = (h % 2) * DK, h // 2
                            ps = sp.tile([P, kw], f32, tag=f"sc{a}")
                            for ck in range(a + 1):
                                nc.tensor.matmul(
                                    ps[:, ck * 512:(ck + 1) * 512],
                                    qT[po:po + DK, pr, a * P:(a + 1) * P],
                                    kT[po:po + DK, pr, ck * 512:(ck + 1) * 512],
                                    start=True, stop=(ck != a))
                            nc.tensor.matmul(ps[:, a * 512:(a + 1) * 512],
                                             ident_r, mask_t, start=False, stop=True)
                            esc = expbuf.tile([P, 1536], bf16, tag="esc")
                            nc.scalar.activation(esc[:, :kw], ps, Act.Exp,
                                                 accum_out=denom[:, a, h:h + 1])
                else:
                    with tc.tile_pool(name="ps3", bufs=2, space="PSUM") as sp:
                        for h in range(H):
                            po, pr = (h % 2) * DK, h // 2
                            pa = sp.tile([P, 1024], f32, tag="sc3a")
                            pb = sp.tile([P, 1024], f32, tag="sc3b")
                            for ck in range(4):
                                tgt = pa if ck < 2 else pb
                                off = (ck % 2) * 512
                                nc.tensor.matmul(
                                    tgt[:, off:off + 512],
                                    qT[po:po + DK, pr, a * P:(a + 1) * P],
                                    kT[po:po + DK, pr, ck * 512:(ck + 1) * 512],
                                    start=True, stop=(ck != 3))
                            nc.tensor.matmul(pb[:, 512:1024], ident_r, mask_t,
                                             start=False, stop=True)
                            esa = expbuf.tile([P, 1024], bf16, tag="esa")
                            nc.scalar.activation(esa, pa, Act.Exp,
                                                 accum_out=denom[:, 3, h:h + 1])
                            esb = expbuf.tile([P, 1024], bf16, tag="esb")
                            nc.scalar.activation(esb, pb, Act.Exp,
                                                 accum_out=d3b[:, h:h + 1])
            nc.vector.tensor_tensor(denom[:, 3, :], denom[:, 3, :], d3b, Alu.add)
            nc.vector.reciprocal(rden[:], denom[:])

            # ============ phase 3: diag probs, attn out, LN1 ============
            with tc.tile_pool(name="pe", bufs=2, space="PSUM") as pe:
                for a in range(NSLOT):
                    ps = pe.tile([P, H], f32, tag="sT")
                    nc.tensor.matmul(ps, sii_eT[:, a * P:(a + 1) * P], ident[:H, :H],
                                     is_transpose=True, start=True, stop=True)
                    nc.vector.tensor_tensor(dp[:, a, :], ps, rden[:, a, :], Alu.mult)
                    nc.vector.tensor_scalar_mul(dp[:, a, :], dp[:, a, :],
                                                keep_t[:, a:a + 1])

                for a in range(NSLOT):
                    wr = stream.tile([P, D], f32, tag="wr")
                    nc.vector.tensor_tensor(
                        wr.rearrange("p (h d) -> p h d", h=H),
                        v_row[:, a, :].rearrange("p (h d) -> p h d", h=H),
                        dp[:, a, :, None].to_broadcast([P, H, DK]), Alu.mult)
                    pw = pe.tile([P, KI, P], f32, tag="pw")
                    for ki in range(KI):
                        nc.tensor.transpose(pw[:, ki, :], wr[:, ki * P:(ki + 1) * P], ident)
                    wTs = stream.tile([P, KI, P], f32r, tag="wTs")
                    nc.vector.tensor_copy(wTs, pw)
                    ps = pe.tile([P, D], f32, tag="po")
                    for ki in range(KI):
                        nc.tensor.matmul(ps, wTs[:, ki, :], wo_t[:, ki, :],
                                         start=(ki == 0), stop=(ki == KI - 1))
                    nc.vector.tensor_tensor(r1[:, a, :], ps, x_row[:, a, :], Alu.add)
                    nc.vector.tensor_tensor(r1[:, a, :], r1[:, a, :], bcasts["bo"], Alu.add)
                    _layernorm(nc, stream, mybir, r1[:, a, :], xn1[:, a, :],
                               bcasts["g1"], bcasts["be1"], eps_t)

                for a in range(NSLOT):
                    pt = pe.tile([P, KI, P], f32, tag="pw")
                    for ki in range(KI):
                        nc.tensor.transpose(pt[:, ki, :],
                                            xn1[:, a, ki * P:(ki + 1) * P], ident)
                    for ki in range(KI):
                        nc.vector.tensor_copy(xnT[:, ki, a * P:(a + 1) * P], pt[:, ki, :])

            # ============ phase 4: FFN, LN2, store ============
            w1_re = W1[:].rearrange("(o p) n -> p o n", p=P)
            w2_re = W2[:].rearrange("(o p) n -> p o n", p=P)
            with (
                tc.tile_pool(name="ph", bufs=2, space="PSUM") as ph,
                tc.tile_pool(name="py", bufs=1, space="PSUM") as py,
            ):
                psy = [py.tile([P, D], f32, tag=f"y{a}", name=f"y{a}") for a in range(NSLOT)]
                for ft in range(NFT):
                    w1c = stream.tile([P, KI, P], f32r, tag="w1c")
                    nc.sync.dma_start(out=w1c, in_=w1_re[:, :, ft * P:(ft + 1) * P])
                    w2c = stream.tile([P, D], f32r, tag="w2c")
                    nc.sync.dma_start(out=w2c, in_=w2_re[:, ft, :])
                    psh = ph.tile([P, NT], f32, tag="h")
                    for ki in range(KI):
                        nc.tensor.matmul(psh, w1c[:, ki, :], xnT[:, ki, :],
                                         start=(ki == 0), stop=(ki == KI - 1))
                    hr = stream.tile([P, NT], f32r, tag="hr")
                    nc.scalar.activation(hr, psh, Act.Relu, bias=b1_t[:, ft:ft + 1])
                    for a in range(NSLOT):
                        nc.tensor.matmul(psy[a], hr[:, a * P:(a + 1) * P], w2c,
                                         start=(ft == 0), stop=(ft == NFT - 1))
                out_re = outv[:].rearrange("(a p) d -> p a d", p=P)
                for a in range(NSLOT):
                    nc.vector.tensor_tensor(out_sb[:, a, :], psy[a], xn1[:, a, :], Alu.add)
                    nc.vector.tensor_tensor(out_sb[:, a, :], out_sb[:, a, :],
                                            bcasts["b2"], Alu.add)
                    _layernorm(nc, stream, mybir, out_sb[:, a, :], out_sb[:, a, :],
                               bcasts["g2"], bcasts["be2"], eps_t)
                    nc.sync.dma_start(out=out_re[:, a, :], in_=out_sb[:, a, :])

    nc.compile()
    return nc


def _get_nc():
    if "nc" not in _CACHE:
        _CACHE["nc"] = _build_nc()
    return _CACHE["nc"]


def kernel(x, lengths, Wq, bq, Wk, bk, Wv, bv, Wo, bo, W1, b1, W2, b2,
           gamma1, beta1, gamma2, beta2):
    global LAST_EXEC_NS
    from concourse.bass_utils import run_bass_kernel_spmd

    x = np.asarray(x, dtype=np.float32)
    lengths = np.asarray(lengths, dtype=np.int32)
    f = lambda a: np.ascontiguousarray(np.asarray(a, dtype=np.float32))

    pad = (np.arange(S)[None, :] < lengths[:, None]).astype(np.float32)
    xm = x * pad[:, :, None]

    # head-pair selector: osel[p, dt, m] = 1 iff 2*dt + p//DK == m
    osel = np.zeros((P, DO, H), dtype=np.float32)
    for dt in range(DO):
        osel[:DK, dt, 2 * dt] = 1.0
        osel[DK:, dt, 2 * dt + 1] = 1.0

    common = dict(Wq=to_f32r(Wq), Wk=to_f32r(Wk), Wv=to_f32r(Wv), Wo=to_f32r(Wo),
                  W1=to_f32r(W1), W2=to_f32r(W2),
                  bq=f(bq), bk=f(bk), bv=f(bv), bo=f(bo), b1=f(b1), b2=f(b2),
                  g1=f(gamma1), be1=f(beta1), g2=f(gamma2), be2=f(beta2),
                  osel=osel, identr=np.eye(P, dtype=np.float32))

    in_maps = []
    for c in range(8):
        b, p = c // 4, c % 4
        xTb = to_f32r(np.ascontiguousarray(xm[b].T))
        cols = np.arange(512)[None, :]
        rows = np.arange(P)[:, None]
        m = to_f32r(np.where(cols <= 4 * rows + p, 0.0, NEG).astype(np.float32))
        tloc = p + 4 * (np.arange(NSLOT)[None, :] * P + rows)
        keep = (tloc < lengths[b]).astype(np.float32)
        in_maps.append(dict(xT=xTb,
                            xTown=np.ascontiguousarray(xTb[:, p::4]),
                            mask=m, keep=keep, **common))

    nc = _get_nc()
    res = run_bass_kernel_spmd(nc, in_maps, list(range(8)), trace=TRACE)
    LAST_EXEC_NS = res.exec_time_ns

    out = np.empty((B, S, D), dtype=np.float32)
    for c in range(8):
        b, p = c // 4, c % 4
        out[b, p::4, :] = res.results[c]["out"]
    return out
